# revision 1
# baseline (speedup 1.0000x reference)
"""Multi-head attention (ViT-style, B=32 N=577 C=768 H=12) on 8 TRN2 NeuronCores.

Sharding: pure data-parallel over batch — each core gets 4 batches plus a
replicated copy of the (host-preprocessed) weights. No collectives.

Per-core pipeline (all matmuls bf16 with fp32 PSUM accumulation):
  stage 1a: QK^T = [Wq*scale; Wk]^T-major matmul  -> qk  [1536, 577] (o-major)
  stage 1b: V'   = x @ Wv^T laid out per-head with a ones column  (for softmax sums)
  stage 2 per head: S^T = K^T.T@Q^T -> exp (no max-sub; scores are O(1)) ->
            O'^T = V'.T @ expS^T  (row 64 = softmax denominators) ->
            reciprocal + partition_broadcast + multiply -> C^T
  stage 3: out = C @ Wp^T + b  (C^T tiles feed matmul lhsT directly)
"""

import sys

sys.path.insert(0, "/opt/trn_rl_repo")

import ml_dtypes
import numpy as np

import concourse.bass as bass  # noqa: F401  (registers AP machinery)
import concourse.mybir as mybir
import concourse.tile as tile
from concourse import bacc, bass_utils

DIM = 768
H = 12
D = 64
N = 577
B = 32
NCORES = 8
BLOC = B // NCORES
SCALE = D**-0.5

BF16 = mybir.dt.bfloat16
F32 = mybir.dt.float32

# token/key chunks along a 577 axis mapped to <=128 partitions
PCH = [(0, 128), (128, 128), (256, 128), (384, 128), (512, 65)]
# free-dim chunks along a 577 axis (<=512 per PSUM bank)
NCH = [(0, 512), (512, 65)]
# free-dim chunks along the 768 output-feature axis
OCH = [(0, 512), (512, 256)]

_NC_CACHE = None


def _build(tc, xT, wqkT, wvT, wpT, bias, out):
    nc = tc.nc
    exp = mybir.ActivationFunctionType.Exp

    with (
        tc.tile_pool(name="w", bufs=1) as wpool,
        tc.tile_pool(name="xb", bufs=2) as xpool,
        tc.tile_pool(name="qk", bufs=2) as qkpool,
        tc.tile_pool(name="vp", bufs=2) as vppool,
        tc.tile_pool(name="es", bufs=2) as espool,
        tc.tile_pool(name="ct", bufs=2) as ctpool,
        tc.tile_pool(name="sm", bufs=4) as smpool,
        tc.tile_pool(name="ob", bufs=3) as opool,
        tc.tile_pool(name="ps", bufs=3, space="PSUM") as pspool,
        tc.tile_pool(name="pst", bufs=2, space="PSUM") as pstpool,
        tc.tile_pool(name="pso", bufs=3, space="PSUM") as psopool,
    ):
        wqk_sb = wpool.tile([128, 6, 2 * DIM], BF16)
        nc.sync.dma_start(wqk_sb[:], wqkT.ap().rearrange("(j p) o -> p j o", p=128))
        wv_sb = wpool.tile([128, 6, DIM], BF16)
        nc.sync.dma_start(wv_sb[:], wvT.ap().rearrange("(j p) o -> p j o", p=128))
        wp_sb = wpool.tile([128, 6, DIM], BF16)
        nc.sync.dma_start(wp_sb[:], wpT.ap().rearrange("(j p) o -> p j o", p=128))
        bias_sb = wpool.tile([128, DIM], F32)
        nc.sync.dma_start(bias_sb[:], bias.ap())

        for b in range(BLOC):
            xt = xpool.tile([128, 6, N], BF16, tag="xt")
            nc.sync.dma_start(xt[:], xT.ap()[b].rearrange("(j p) t -> p j t", p=128))

            # ---- stage 1a: QK^T [1536, 577], o-tile j covers rows 128j..128j+127.
            # Order tiles so head h's Q (tile h//2) and K (tile 6+h//2) land early.
            qk = qkpool.tile([128, 12, N], BF16, tag="qk")
            for j in [0, 6, 1, 7, 2, 8, 3, 9, 4, 10, 5, 11]:
                for n0, nsz in NCH:
                    ps = pspool.tile([128, 512], F32, tag="mm")
                    for c in range(6):
                        nc.tensor.matmul(
                            ps[:, :nsz],
                            wqk_sb[:, c, j * 128 : (j + 1) * 128],
                            xt[:, c, n0 : n0 + nsz],
                            start=(c == 0),
                            stop=(c == 5),
                        )
                    nc.vector.tensor_copy(qk[:, j, n0 : n0 + nsz], ps[:, :nsz])

            # ---- stage 1b: V' [577, 12*65]: per head 64 value cols + a ones col.
            vp = vppool.tile([128, 5, H * 65], BF16, tag="vp")
            for it in range(5):
                ones_col = vp[:, it].rearrange("p (h c) -> p h c", c=65)[:, :, 64:65]
                nc.gpsimd.memset(ones_col, 1.0)
            for it, (t0, tsz) in enumerate(PCH):
                for o0, osz in OCH:
                    ps = pspool.tile([128, 512], F32, tag="mm")
                    for c in range(6):
                        nc.tensor.matmul(
                            ps[:tsz, :osz],
                            xt[:, c, t0 : t0 + tsz],
                            wv_sb[:, c, o0 : o0 + osz],
                            start=(c == 0),
                            stop=(c == 5),
                        )
                    nh = osz // D
                    h0 = o0 // D
                    src = ps[:tsz, :osz].rearrange("p (h d) -> p h d", d=D)
                    dst = vp[:tsz, it].rearrange("p (h c) -> p h c", c=65)[
                        :, h0 : h0 + nh, 0:D
                    ]
                    nc.vector.tensor_copy(dst, src)

            # ---- stage 2: per-head attention
            ct = ctpool.tile([128, 6, N], BF16, tag="ct")
            for h in range(H):
                jq = h // 2
                pq = (h % 2) * 64
                es = espool.tile([128, 5, N], BF16, tag="es")
                # scores S^T (keys on partitions, queries on free) + exp
                for mc, (m0, msz) in enumerate(PCH):
                    ps = pspool.tile([128, 512], F32, tag="mm")
                    nc.tensor.matmul(
                        ps[:msz, :512],
                        qk[pq : pq + 64, 6 + jq, m0 : m0 + msz],
                        qk[pq : pq + 64, jq, 0:512],
                        start=True,
                        stop=True,
                    )
                    nc.scalar.activation(es[:msz, mc, 0:512], ps[:msz, :512], exp)
                # the 65-wide query tail: all 5 key-chunks share one PSUM bank
                pst = pstpool.tile([128, 5, 65], F32, tag="tail")
                for mc, (m0, msz) in enumerate(PCH):
                    nc.tensor.matmul(
                        pst[:msz, mc, :],
                        qk[pq : pq + 64, 6 + jq, m0 : m0 + msz],
                        qk[pq : pq + 64, jq, 512:577],
                        start=True,
                        stop=True,
                    )
                nc.scalar.activation(es[:, :, 512:577], pst[:, :, :], exp)

                # O'^T = V'.T @ expS^T ; row 64 = per-query softmax denominator
                psos = []
                for n0, nsz in NCH:
                    pso = psopool.tile([65, 512], F32, tag="pv")
                    for mc, (m0, msz) in enumerate(PCH):
                        nc.tensor.matmul(
                            pso[:65, :nsz],
                            vp[:msz, mc, h * 65 : (h + 1) * 65],
                            es[:msz, mc, n0 : n0 + nsz],
                            start=(mc == 0),
                            stop=(mc == 4),
                        )
                    psos.append(pso)

                recip = smpool.tile([1, N], F32, tag="recip")
                nc.vector.reciprocal(recip[0:1, 0:512], psos[0][64:65, 0:512])
                nc.vector.reciprocal(recip[0:1, 512:577], psos[1][64:65, 0:65])
                rb = smpool.tile([64, N], F32, tag="rb")
                nc.gpsimd.partition_broadcast(rb[:], recip[:])

                nc.vector.tensor_mul(
                    ct[pq : pq + 64, jq, 0:512], psos[0][0:64, 0:512], rb[0:64, 0:512]
                )
                nc.vector.tensor_mul(
                    ct[pq : pq + 64, jq, 512:577], psos[1][0:64, 0:65], rb[0:64, 512:577]
                )

            # ---- stage 3: out = C @ Wp^T + b
            for t0, tsz in PCH:
                ob = opool.tile([128, DIM], F32, tag="ob")
                for o0, osz in OCH:
                    ps = pspool.tile([128, 512], F32, tag="mm")
                    for c in range(6):
                        nc.tensor.matmul(
                            ps[:tsz, :osz],
                            ct[:, c, t0 : t0 + tsz],
                            wp_sb[:, c, o0 : o0 + osz],
                            start=(c == 0),
                            stop=(c == 5),
                        )
                    nc.vector.tensor_add(
                        ob[:tsz, o0 : o0 + osz],
                        ps[:tsz, :osz],
                        bias_sb[:tsz, o0 : o0 + osz],
                    )
                nc.sync.dma_start(out.ap()[b, t0 : t0 + tsz, :], ob[:tsz, :])


def _build_nc():
    global _NC_CACHE
    if _NC_CACHE is not None:
        return _NC_CACHE
    nc = bacc.Bacc("TRN2", target_bir_lowering=False, debug=False)
    xT = nc.dram_tensor("xT", [BLOC, DIM, N], BF16, kind="ExternalInput")
    wqkT = nc.dram_tensor("wqkT", [DIM, 2 * DIM], BF16, kind="ExternalInput")
    wvT = nc.dram_tensor("wvT", [DIM, DIM], BF16, kind="ExternalInput")
    wpT = nc.dram_tensor("wpT", [DIM, DIM], BF16, kind="ExternalInput")
    bias = nc.dram_tensor("bias", [128, DIM], F32, kind="ExternalInput")
    out = nc.dram_tensor("out", [BLOC, N, DIM], F32, kind="ExternalOutput")
    with tile.TileContext(nc) as tc:
        _build(tc, xT, wqkT, wvT, wpT, bias, out)
    nc.compile()
    _NC_CACHE = nc
    return nc


def _prep_inputs(x, W_qkv, W_proj, b_proj):
    bf = ml_dtypes.bfloat16
    x = np.asarray(x, dtype=np.float32)
    W_qkv = np.asarray(W_qkv, dtype=np.float32)
    W_proj = np.asarray(W_proj, dtype=np.float32)
    b_proj = np.asarray(b_proj, dtype=np.float32)

    wq = W_qkv[:DIM] * np.float32(SCALE)
    wk = W_qkv[DIM : 2 * DIM]
    wv = W_qkv[2 * DIM :]
    wqkT = np.ascontiguousarray(np.concatenate([wq, wk], axis=0).T).astype(bf)
    wvT = np.ascontiguousarray(wv.T).astype(bf)
    wpT = np.ascontiguousarray(W_proj.T).astype(bf)
    bias_bc = np.ascontiguousarray(np.broadcast_to(b_proj, (128, DIM))).astype(
        np.float32
    )

    in_maps = []
    for c in range(NCORES):
        xb = x[c * BLOC : (c + 1) * BLOC]  # [BLOC, N, DIM]
        xT = np.ascontiguousarray(xb.transpose(0, 2, 1)).astype(bf)
        in_maps.append(
            {"xT": xT, "wqkT": wqkT, "wvT": wvT, "wpT": wpT, "bias": bias_bc}
        )
    return in_maps


def _run(x, W_qkv, W_proj, b_proj, trace=False):
    nc = _build_nc()
    in_maps = _prep_inputs(x, W_qkv, W_proj, b_proj)
    res = bass_utils.run_bass_kernel_spmd(
        nc, in_maps, core_ids=list(range(NCORES)), trace=trace
    )
    out = np.concatenate(
        [np.asarray(res.results[c]["out"], dtype=np.float32) for c in range(NCORES)],
        axis=0,
    )
    return out, res


def kernel(x, W_qkv, W_proj, b_proj):
    out, _ = _run(x, W_qkv, W_proj, b_proj, trace=False)
    return out


# revision 9
# speedup vs baseline: 1.2798x; 1.2798x over previous
"""Multi-head attention (ViT-style, B=32 N=577 C=768 H=12) on 8 TRN2 NeuronCores.

Sharding: pure data-parallel over batch — each core gets 4 batches plus a
replicated copy of the (host-preprocessed) weights. No collectives.

Per-core pipeline (all matmuls bf16 with fp32 PSUM accumulation):
  stage 1a: QK^T = [Wq*scale; Wk]^T-major matmul  -> qk  [1536, 577] (o-major)
  stage 1b: V'   = x @ Wv^T laid out per-head with a ones column  (for softmax sums)
  stage 2 per head: S^T = K^T.T@Q^T -> exp (no max-sub; scores are O(1)) ->
            O'^T = V'.T @ expS^T  (row 64 = softmax denominators) ->
            reciprocal + partition_broadcast + multiply -> C^T
  stage 3: out = C @ Wp^T + b  (C^T tiles feed matmul lhsT directly)
"""

import sys

sys.path.insert(0, "/opt/trn_rl_repo")

import ml_dtypes
import numpy as np

import concourse.bass as bass  # noqa: F401  (registers AP machinery)
import concourse.mybir as mybir
import concourse.tile as tile
from concourse import bacc, bass_utils

DIM = 768
H = 12
D = 64
N = 577
B = 32
NCORES = 8
BLOC = B // NCORES
SCALE = D**-0.5

BF16 = mybir.dt.bfloat16
F32 = mybir.dt.float32

# token/key chunks along a 577 axis mapped to <=128 partitions
PCH = [(0, 128), (128, 128), (256, 128), (384, 128), (512, 65)]
# free-dim chunks along a 577 axis (<=512 per PSUM bank)
NCH = [(0, 512), (512, 65)]
# free-dim chunks along the 768 output-feature axis
OCH = [(0, 512), (512, 256)]

_NC_CACHE = None


def _build(tc, xT, wqkT, wvT, wpT, bias, out):
    nc = tc.nc
    exp = mybir.ActivationFunctionType.Exp

    with (
        tc.tile_pool(name="w", bufs=1) as wpool,
        tc.tile_pool(name="xb", bufs=2) as xpool,
        tc.tile_pool(name="qk", bufs=2) as qkpool,
        tc.tile_pool(name="vp", bufs=2) as vppool,
        tc.tile_pool(name="es", bufs=4) as espool,
        tc.tile_pool(name="ct", bufs=2) as ctpool,
        tc.tile_pool(name="sm", bufs=4) as smpool,
        tc.tile_pool(name="ob", bufs=3) as opool,
        tc.tile_pool(name="ps", bufs=3, space="PSUM") as pspool,
        tc.tile_pool(name="pst", bufs=2, space="PSUM") as pstpool,
        tc.tile_pool(name="pso", bufs=3, space="PSUM") as psopool,
    ):
        # Split the big weight loads per contraction-chunk so the first
        # matmuls can start as soon as chunk 0 lands.
        wqk_sb = wpool.tile([128, 6, 2 * DIM], BF16)
        wqk_dr = wqkT.ap().rearrange("(j p) o -> p j o", p=128)
        wv_sb = wpool.tile([128, 6, DIM], BF16)
        wv_dr = wvT.ap().rearrange("(j p) o -> p j o", p=128)
        wp_sb = wpool.tile([128, 6, DIM], BF16)
        wp_dr = wpT.ap().rearrange("(j p) o -> p j o", p=128)
        for c in range(6):
            nc.sync.dma_start(wqk_sb[:, c], wqk_dr[:, c])
            nc.sync.dma_start(wv_sb[:, c], wv_dr[:, c])
        for c in range(6):
            nc.sync.dma_start(wp_sb[:, c], wp_dr[:, c])
        bias_sb = wpool.tile([128, DIM], F32)
        nc.sync.dma_start(bias_sb[:], bias.ap())

        for b in range(BLOC):
            xt = xpool.tile([128, 6, N], BF16, tag="xt")
            xt_dr = xT.ap()[b].rearrange("(j p) t -> p j t", p=128)
            for c in range(6):
                nc.sync.dma_start(xt[:, c], xt_dr[:, c])

            # ---- stage 1a: QK^T [1536, 577], o-tile j covers rows 128j..128j+127.
            # Order tiles so head h's Q (tile h//2) and K (tile 6+h//2) land early.
            qk = qkpool.tile([128, 12, N], BF16, tag="qk")
            for j in [0, 6, 1, 7, 2, 8, 3, 9, 4, 10, 5, 11]:
                for n0, nsz in NCH:
                    ps = pspool.tile([128, 512], F32, tag="mm")
                    for c in range(6):
                        nc.tensor.matmul(
                            ps[:, :nsz],
                            wqk_sb[:, c, j * 128 : (j + 1) * 128],
                            xt[:, c, n0 : n0 + nsz],
                            start=(c == 0),
                            stop=(c == 5),
                        )
                    nc.vector.tensor_copy(qk[:, j, n0 : n0 + nsz], ps[:, :nsz])

            # ---- stage 1b: V' [577, 12*65]: per head 64 value cols + a ones col.
            vp = vppool.tile([128, 5, H * 65], BF16, tag="vp")
            for it in range(5):
                ones_col = vp[:, it].rearrange("p (h c) -> p h c", c=65)[:, :, 64:65]
                nc.gpsimd.memset(ones_col, 1.0)
            for it, (t0, tsz) in enumerate(PCH):
                for o0, osz in OCH:
                    ps = pspool.tile([128, 512], F32, tag="mm")
                    for c in range(6):
                        nc.tensor.matmul(
                            ps[:tsz, :osz],
                            xt[:, c, t0 : t0 + tsz],
                            wv_sb[:, c, o0 : o0 + osz],
                            start=(c == 0),
                            stop=(c == 5),
                        )
                    nh = osz // D
                    h0 = o0 // D
                    src = ps[:tsz, :osz].rearrange("p (h d) -> p h d", d=D)
                    dst = vp[:tsz, it].rearrange("p (h c) -> p h c", c=65)[
                        :, h0 : h0 + nh, 0:D
                    ]
                    nc.vector.tensor_copy(dst, src)

            # ---- stage 2: per-head attention, software-pipelined so the PE
            # never waits on the exp (ACT) results of the head it just scored.
            ct = ctpool.tile([128, 6, N], BF16, tag="ct")
            es_tiles = [None] * H

            def scores(h):
                jq = h // 2
                pq = (h % 2) * 64
                es = espool.tile([128, 5, N], BF16, tag="es")
                es_tiles[h] = es
                kT = qk[pq : pq + 64, 6 + jq]
                qT = qk[pq : pq + 64, jq]
                for mc, (m0, msz) in enumerate(PCH):
                    ps = pspool.tile([128, 512], F32, tag="mm")
                    nc.tensor.matmul(
                        ps[:msz, :512],
                        kT[:, m0 : m0 + msz],
                        qT[:, 0:512],
                        start=True,
                        stop=True,
                    )
                    nc.scalar.activation(es[:msz, mc, 0:512], ps[:msz, :512], exp)
                # the 65-wide query tail: all 5 key-chunks share one PSUM bank
                pst = pstpool.tile([128, 5, 65], F32, tag="tail")
                for mc, (m0, msz) in enumerate(PCH):
                    nc.tensor.matmul(
                        pst[:msz, mc, :],
                        kT[:, m0 : m0 + msz],
                        qT[:, 512:577],
                        start=True,
                        stop=True,
                    )
                nc.scalar.activation(es[:, :, 512:577], pst[:, :, :], exp)

            def pv_norm(h):
                jq = h // 2
                pq = (h % 2) * 64
                es = es_tiles[h]
                # O'^T = V'.T @ expS^T ; row 64 = per-query softmax denominator
                psos = []
                for n0, nsz in NCH:
                    pso = psopool.tile([65, 512], F32, tag="pv")
                    for mc, (m0, msz) in enumerate(PCH):
                        nc.tensor.matmul(
                            pso[:65, :nsz],
                            vp[:msz, mc, h * 65 : (h + 1) * 65],
                            es[:msz, mc, n0 : n0 + nsz],
                            start=(mc == 0),
                            stop=(mc == 4),
                        )
                    psos.append(pso)

                # custom-DVE recip mis-reads PSUM; bounce the sums row to SBUF
                sums = smpool.tile([1, N], F32, tag="sums")
                nc.vector.tensor_copy(sums[0:1, 0:512], psos[0][64:65, 0:512])
                nc.vector.tensor_copy(sums[0:1, 512:577], psos[1][64:65, 0:65])
                recip = smpool.tile([1, N], F32, tag="recip")
                nc.vector.reciprocal_approx_fast(recip[:], sums[:])
                rb = smpool.tile([64, N], F32, tag="rb")
                nc.gpsimd.partition_broadcast(rb[:], recip[:])

                nc.vector.tensor_mul(
                    ct[pq : pq + 64, jq, 0:512], psos[0][0:64, 0:512], rb[0:64, 0:512]
                )
                nc.vector.tensor_mul(
                    ct[pq : pq + 64, jq, 512:577], psos[1][0:64, 0:65], rb[0:64, 512:577]
                )
                es_tiles[h] = None

            LOOK = 3
            for h in range(H):
                scores(h)
                if h >= LOOK:
                    pv_norm(h - LOOK)
            for h in range(H - LOOK, H):
                pv_norm(h)

            # ---- stage 3: out = C @ Wp^T + b
            for t0, tsz in PCH:
                ob = opool.tile([128, DIM], F32, tag="ob")
                for o0, osz in OCH:
                    ps = pspool.tile([128, 512], F32, tag="mm")
                    for c in range(6):
                        nc.tensor.matmul(
                            ps[:tsz, :osz],
                            ct[:, c, t0 : t0 + tsz],
                            wp_sb[:, c, o0 : o0 + osz],
                            start=(c == 0),
                            stop=(c == 5),
                        )
                    nc.vector.tensor_add(
                        ob[:tsz, o0 : o0 + osz],
                        ps[:tsz, :osz],
                        bias_sb[:tsz, o0 : o0 + osz],
                    )
                nc.sync.dma_start(out.ap()[b, t0 : t0 + tsz, :], ob[:tsz, :])


def _build_nc():
    global _NC_CACHE
    if _NC_CACHE is not None:
        return _NC_CACHE
    nc = bacc.Bacc("TRN2", target_bir_lowering=False, debug=False)
    xT = nc.dram_tensor("xT", [BLOC, DIM, N], BF16, kind="ExternalInput")
    wqkT = nc.dram_tensor("wqkT", [DIM, 2 * DIM], BF16, kind="ExternalInput")
    wvT = nc.dram_tensor("wvT", [DIM, DIM], BF16, kind="ExternalInput")
    wpT = nc.dram_tensor("wpT", [DIM, DIM], BF16, kind="ExternalInput")
    bias = nc.dram_tensor("bias", [128, DIM], F32, kind="ExternalInput")
    out = nc.dram_tensor("out", [BLOC, N, DIM], F32, kind="ExternalOutput")
    with tile.TileContext(nc) as tc:
        _build(tc, xT, wqkT, wvT, wpT, bias, out)
    nc.compile()
    _NC_CACHE = nc
    return nc


def _prep_inputs(x, W_qkv, W_proj, b_proj):
    bf = ml_dtypes.bfloat16
    x = np.asarray(x, dtype=np.float32)
    W_qkv = np.asarray(W_qkv, dtype=np.float32)
    W_proj = np.asarray(W_proj, dtype=np.float32)
    b_proj = np.asarray(b_proj, dtype=np.float32)

    wq = W_qkv[:DIM] * np.float32(SCALE)
    wk = W_qkv[DIM : 2 * DIM]
    wv = W_qkv[2 * DIM :]
    wqkT = np.ascontiguousarray(np.concatenate([wq, wk], axis=0).T).astype(bf)
    wvT = np.ascontiguousarray(wv.T).astype(bf)
    wpT = np.ascontiguousarray(W_proj.T).astype(bf)
    bias_bc = np.ascontiguousarray(np.broadcast_to(b_proj, (128, DIM))).astype(
        np.float32
    )

    in_maps = []
    for c in range(NCORES):
        xb = x[c * BLOC : (c + 1) * BLOC]  # [BLOC, N, DIM]
        xTc = np.ascontiguousarray(xb.transpose(0, 2, 1)).astype(bf)
        in_maps.append(
            {"xT": xTc, "wqkT": wqkT, "wvT": wvT, "wpT": wpT, "bias": bias_bc}
        )
    return in_maps


def _run(x, W_qkv, W_proj, b_proj, trace=False):
    nc = _build_nc()
    in_maps = _prep_inputs(x, W_qkv, W_proj, b_proj)
    res = bass_utils.run_bass_kernel_spmd(
        nc, in_maps, core_ids=list(range(NCORES)), trace=trace
    )
    out = np.concatenate(
        [np.asarray(res.results[c]["out"], dtype=np.float32) for c in range(NCORES)],
        axis=0,
    )
    return out, res


def kernel(x, W_qkv, W_proj, b_proj):
    out, _ = _run(x, W_qkv, W_proj, b_proj, trace=False)
    return out
